# revision 19
# baseline (speedup 1.0000x reference)
"""Multi-head causal attention (scores = K @ Q^T variant) on 8 TRN2 NeuronCores.

Head-parallel sharding: core c computes heads (2c, 2c+1) end-to-end and the
host concatenates the per-core [T, 128] outputs along the feature axis.

The baseline was Scalar(ACT)-bound: one exp per 128-key j-block paced the
whole steady state at ~1.07us, with the 2-deep score-PSUM ring
(score(jb+2) waits exp(jb)) locking the PE to the exp engine.  This version
splits the softmax exp across engines in strictly alternating key-block
pairs and uses fp8 DoubleRow for the ACT-path AV matmuls:

  - "f8" pairs (alternating off-diagonal): ACT exp emits fp8e4 directly
    (exp(s/8 - OFF), OFF=2 keeps exp below the e4m3 max; the offset divides
    out of the softmax ratio exactly).  The AV contraction runs as ONE
    DoubleRow matmul per head per 256-key pair (2 fp8 weights/cell, virtual
    256-deep contraction) instead of 4 bf16 matmuls.
  - "dve" pairs (the other off-diagonal half + first diagonal pair): the
    Vector engine computes a Schraudolph fast-exp: int16((s + B/A) * A) IS
    the bf16 bit pattern of e^(s/8-OFF) with a piecewise-linear mantissa
    (~+-3% after centering, cancels in the softmax ratio).  On diagonal
    blocks the causal mask folds into the same instruction: the in1 scale
    tensor holds A where kept and 0 where masked, so masked lanes emit
    int16 0 == bf16 +0.0.
  - "a16" (second diagonal pair + all of i-block 0): exact ACT exp in bf16
    + mask multiply — rows with few keys get no sawtooth/quantization error
    (they lack the averaging that damps it elsewhere).

The strict dve/ACT pair alternation gives each exp engine a two-pair
(~3.8us) window per ~2.2us of exp work, so the PSUM ring never binds; AV
matmuls are emitted two pairs behind the scores in whole-pair groups, which
keeps same-geometry matmuls back-to-back on the PE (only the first matmul
after a weight-geometry change pays the ~110ns LDWEIGHTS exposure).

Numerics (numpy sim vs fp64 reference, matches HW): rel absmax ~1.0e-2 vs
the 2e-2 gate.  Measured: ~196-199us vs the 214.6us bf16 baseline.

Other notes (unchanged from baseline): x arrives transposed/bf16; Q^T/K^T
live [128, T] with the two heads on disjoint 64-row PE groups so their score
matmuls run concurrently; scores are computed transposed so AV contracts on
the partition axis; V carries a fused ones-column so one matmul yields both
the weighted sum and the softmax denominator; the host does the final divide
and un-transpose during the gather.
"""

import numpy as np

T, D, H, HS = 4096, 1024, 16, 64
NCORES = 8
HPC = H // NCORES  # heads per core = 2
DC = D // 128      # 8 contraction chunks
TC = T // 512      # 8 t-chunks for projections
JBN = T // 128     # 32 j-blocks (128 keys each)
VF_W = 80          # fp8 V row stride (65 used; 80 keeps DoubleRow step%16==0)

OFF = 2.0                       # exp offset: exp(s/8 - OFF); softmax-invariant
LOG2E = 1.4426950408889634
FE_A = 128.0 * LOG2E / 8.0      # fast-exp scale on raw scores
FE_C = 0.0430                   # centers the PWL sawtooth (half of 0.0861)
FE_B = 128.0 * (127.0 - FE_C - OFF * LOG2E)

_cached_nc = None


def _kind(ib, p):
    """Exp path for key-block pair p (j-blocks 2p, 2p+1) of i-block ib.

    Pairs strictly alternate between the DVE fast-exp path and the ACT
    paths (fp8 off-diagonal / exact-bf16 a16): the 2-deep score-PSUM ring
    makes score(jb+2) wait on exp(jb), so two consecutive same-engine pairs
    lock the pipeline to that engine's ~1.1us-per-jb exp latency, while
    alternation gives each engine a two-pair window per pair of exps.
    Sequence per block: dve, f8, dve, f8, ..., f8, dve(diag), a16(diag).
    ib0 is entirely exact (a16) to protect the few-key early rows.
    """
    npair = 2 * (ib + 1)
    if ib == 0:
        return "a16"
    if p == npair - 2:
        return "dve"   # diagonal blocks q=0,1: mask folds into the fast-exp
    if p == npair - 1:
        return "a16"   # diagonal blocks q=2,3: exact exp + mask multiply
    return "f8" if p % 2 == 1 else "dve"


def _emit(tc, nc, xT, w6, out):
    import concourse.bass as bass  # noqa: F401
    import concourse.mybir as mybir

    f32 = mybir.dt.float32
    bf16 = mybir.dt.bfloat16
    fp8 = mybir.dt.float8e4
    i16 = mybir.dt.int16
    Exp = mybir.ActivationFunctionType.Exp
    CopyF = mybir.ActivationFunctionType.Copy
    ne = mybir.AluOpType.not_equal
    ge = mybir.AluOpType.is_ge
    add = mybir.AluOpType.add
    mult = mybir.AluOpType.mult
    DR = mybir.MatmulPerfMode.DoubleRow

    with (
        tc.tile_pool(name="const", bufs=1) as constp,
        tc.tile_pool(name="wpool", bufs=1) as wpool,
        tc.tile_pool(name="bigp", bufs=1) as bigp,
        tc.tile_pool(name="xpool", bufs=3) as xpool,
        tc.tile_pool(name="vtp", bufs=2) as vtp,
        tc.tile_pool(name="esb", bufs=12) as esbp,
        tc.tile_pool(name="esf", bufs=6) as esfp,
        tc.tile_pool(name="finp", bufs=4) as finp,
        # PSUM budget (8 banks): s 2x2 + o 2x1 + p 2x1 (p shared with the
        # V-transpose tiles).
        tc.tile_pool(name="sp", bufs=2, space="PSUM") as sp,
        tc.tile_pool(name="op", bufs=2, space="PSUM") as op,
        tc.tile_pool(name="pp", bufs=2, space="PSUM") as pp,
    ):
        # ---- input DMAs for weights + first x chunk go first ------------
        w6r = w6.rearrange("(dc p) f -> p dc f", p=128)
        xTr = xT.rearrange("(dc p) t -> p dc t", p=128)
        w6sb = wpool.tile([128, DC, 6 * HS], bf16)
        xts = []
        xt0 = xpool.tile([128, DC, 512], bf16, tag="xt", name="xt0")
        nc.sync.dma_start(out=w6sb[:, 0, :], in_=w6r[:, 0, :])
        nc.sync.dma_start(out=xt0[:, 0, :], in_=xTr[:, 0, 0:512])
        nc.sync.dma_start(out=w6sb[:, 1:DC, :], in_=w6r[:, 1:DC, :])
        for dc in range(1, DC):
            nc.sync.dma_start(out=xt0[:, dc, :], in_=xTr[:, dc, 0:512])
        xts.append(xt0)

        # ---- constants (gpsimd; overlaps the DMAs) ----------------------
        id64 = constp.tile([128, 64], bf16)
        nc.gpsimd.memset(id64, 0.0)
        nc.gpsimd.affine_select(
            out=id64, in_=id64, compare_op=ne, fill=1.0,
            base=0, channel_multiplier=1, pattern=[[-1, 64]],
        )
        nc.gpsimd.affine_select(
            out=id64, in_=id64, compare_op=ne, fill=1.0,
            base=-64, channel_multiplier=1, pattern=[[-1, 64]],
        )
        # bf16 multiplicative causal masks (a16 path): keep iff il >= jl + 128q.
        mask4 = constp.tile([128, 4, 512], bf16)
        for q in range(4):
            nc.gpsimd.memset(mask4[:, q, :], 1.0)
            nc.gpsimd.affine_select(
                out=mask4[:, q, :], in_=mask4[:, q, :], compare_op=ge, fill=0.0,
                base=-128 * q, channel_multiplier=-1, pattern=[[1, 512]],
            )
        # fp32 fast-exp scale masks (dve path): FE_A where kept, 0 masked.
        amask4 = constp.tile([128, 4, HPC, 512], f32)
        for q in range(4):
            nc.gpsimd.memset(amask4[:, q, :, :], FE_A)
            for h in range(HPC):
                nc.gpsimd.affine_select(
                    out=amask4[:, q, h, :], in_=amask4[:, q, h, :],
                    compare_op=ge, fill=0.0,
                    base=-128 * q, channel_multiplier=-1, pattern=[[1, 512]],
                )

        # ---- persistent activations ------------------------------------
        QT = bigp.tile([128, T], bf16)   # head0 rows 0-63, head1 rows 64-127
        KT = bigp.tile([128, T], bf16)
        Vb = [bigp.tile([128, JBN, HS + 1], bf16, name=f"Vb{h}") for h in range(HPC)]
        # fp8 V for the DoubleRow AV pairs: slot [p, par] holds key-block
        # 2p + par, so any pair p can run the fp8 path.
        Vf = [bigp.tile([128, JBN // 2, 2, VF_W], fp8, name=f"Vf{h}") for h in range(HPC)]
        onesb = constp.tile([128, JBN], bf16)
        nc.gpsimd.memset(onesb, 1.0)
        for h in range(HPC):
            nc.vector.tensor_copy(Vb[h][:, :, HS], onesb)
            nc.gpsimd.tensor_copy(Vf[h][:, :, :, HS], onesb)
        boff = constp.tile([128, 1], f32)  # activation bias: exp(s/8 - OFF)
        nc.gpsimd.memset(boff, -OFF)

        def emit_proj_chunk(tcj):
            ts = slice(tcj * 512, (tcj + 1) * 512)
            if tcj + 1 < TC:  # prefetch next x chunk (batched 3D DMAs)
                nxt = slice((tcj + 1) * 512, (tcj + 2) * 512)
                xtn = xpool.tile([128, DC, 512], bf16, tag="xt", name=f"xt{tcj + 1}")
                nc.sync.dma_start(out=xtn[:, 0:4, :], in_=xTr[:, 0:4, nxt])
                nc.sync.dma_start(out=xtn[:, 4:8, :], in_=xTr[:, 4:8, nxt])
                xts.append(xtn)
            xt = xts[tcj]
            for fc, dest in ((0, QT), (1, KT)):
                ps = pp.tile([128, 512], f32, tag="p", name=f"ps{fc}_{tcj}")
                for dc in range(DC):
                    nc.tensor.matmul(
                        ps,
                        lhsT=w6sb[:, dc, fc * 128:(fc + 1) * 128],
                        rhs=xt[:, dc, :],
                        start=(dc == 0), stop=(dc == DC - 1),
                    )
                nc.scalar.activation(dest[:, ts], ps, CopyF, scale=1.0)
            psv = pp.tile([128, 512], f32, tag="p", name=f"psv_{tcj}")
            for dc in range(DC):
                nc.tensor.matmul(
                    psv,
                    lhsT=w6sb[:, dc, 256:384],
                    rhs=xt[:, dc, :],
                    start=(dc == 0), stop=(dc == DC - 1),
                )
            vts = vtp.tile([128, 512], bf16, tag="vts", name=f"vts_{tcj}")
            nc.vector.tensor_copy(vts, psv)
            for q in range(4):
                jb = tcj * 4 + q
                for h in range(HPC):
                    ptv = pp.tile([128, 64], bf16, tag="p", name=f"ptv{h}_{tcj}_{q}")
                    nc.tensor.transpose(
                        ptv,
                        in_=vts[h * 64:(h + 1) * 64, q * 128:(q + 1) * 128],
                        identity=id64[h * 64:(h + 1) * 64, :],
                        tile_position=(h * 64, 0),
                    )
                    nc.vector.tensor_copy(Vb[h][:, jb, 0:HS], ptv)
                    nc.gpsimd.tensor_copy(
                        Vf[h][:, jb // 2, jb % 2, 0:HS],
                        Vb[h][:, jb, 0:HS],
                    )

        def emit_attn_block(ib):
            isl = slice(ib * 512, (ib + 1) * 512)
            npair = 2 * (ib + 1)
            po = [
                op.tile([HS + 1, 512], f32, tag="o", name=f"po{h}_{ib}")
                for h in range(HPC)
            ]

            def emit_av(p, kind, es, first, last):
                if kind == "f8":
                    for h in range(HPC):
                        nc.tensor.matmul(
                            po[h],
                            lhsT=Vf[h][:, p, :, 0:HS + 1],
                            rhs=es[:, :, h, :],
                            start=first, stop=last,
                            perf_mode=DR,
                        )
                else:
                    for t in range(2):
                        for h in range(HPC):
                            nc.tensor.matmul(
                                po[h],
                                lhsT=Vb[h][:, 2 * p + t, :],
                                rhs=es[t][:, h, :],
                                start=(first and t == 0),
                                stop=(last and t == 1),
                            )

            pending = []
            for p in range(npair):
                kind = _kind(ib, p)
                if kind == "f8":
                    es = esfp.tile(
                        [128, 2, HPC, 512], fp8, tag="es8", name=f"es8_{ib}_{p}"
                    )
                else:
                    es = [
                        esbp.tile([128, HPC, 512], bf16, tag="esb",
                                  name=f"esb_{ib}_{p}_{t}")
                        for t in range(2)
                    ]
                for t in range(2):
                    jb = 2 * p + t
                    ps = sp.tile([128, HPC, 512], f32, tag="s",
                                 name=f"s_{ib}_{jb}")
                    for h in range(HPC):
                        nc.tensor.matmul(
                            ps[:, h, :],
                            lhsT=QT[h * 64:(h + 1) * 64, jb * 128:(jb + 1) * 128],
                            rhs=KT[h * 64:(h + 1) * 64, isl],
                            start=True, stop=True,
                            tile_position=(h * 64, 0),
                        )
                    q = jb - 4 * ib  # diagonal offset (>=0 on diagonal blocks)
                    if kind == "f8":
                        nc.scalar.activation(
                            es[:, t, :, :], ps, Exp,
                            scale=float(1.0 / np.sqrt(HS)), bias=boff,
                        )
                    elif kind == "a16":
                        nc.scalar.activation(
                            es[t], ps, Exp,
                            scale=float(1.0 / np.sqrt(HS)), bias=boff,
                        )
                        if q >= 0:
                            for h in range(HPC):
                                nc.vector.tensor_mul(
                                    es[t][:, h, :], es[t][:, h, :], mask4[:, q, :]
                                )
                    else:  # dve fast-exp; mask folds in via amask4
                        eo = es[t].bitcast(i16)
                        if q >= 0:
                            nc.vector.scalar_tensor_tensor(
                                out=eo, in0=ps, scalar=FE_B / FE_A,
                                in1=amask4[:, q, :, :], op0=add, op1=mult,
                            )
                        else:
                            nc.vector.tensor_scalar(
                                eo, ps, FE_B / FE_A, FE_A, add, mult
                            )
                pending.append((p, kind, es, p == 0, p == npair - 1))
                if len(pending) > 2:
                    emit_av(*pending.pop(0))
            for item in pending:
                emit_av(*item)
            # evacuate the unnormalized O^T + denominator row; the host does
            # the (tiny) divide and the un-transpose during the gather.
            for h in range(HPC):
                ot = finp.tile([HS + 1, 512], f32, tag="ot", name=f"ot{h}_{ib}")
                nc.scalar.activation(ot, po[h], CopyF, scale=1.0)
                nc.sync.dma_start(
                    out=out[h * (HS + 1):(h + 1) * (HS + 1), isl], in_=ot
                )

        # Staircase: attention block k only depends on projection chunks <= k.
        for k in range(TC):
            emit_proj_chunk(k)
            emit_attn_block(k)


# walrus engine-instruction encodings have a single sync-wait slot; hoist
# extra waits onto per-wait NoOps for everything except generated NoOps.
_NO_HOIST_TYPES = frozenset({"InstNoOp"})


def _pair_ldweights(nc):
    """Reorder LDW0,MM0,LDW1,MM1 -> LDW0,LDW1,MM0,MM1 for row-group pairs.

    When the second weight load targets PE rows 64-127 while the first
    matmul only occupies rows 0-63, the loads run concurrently on disjoint
    sub-arrays and both matmul streams overlap, instead of serializing the
    second load behind the first stream.
    """
    for f in nc.m.functions:
        for blk in f.blocks:
            insts = blk.instructions
            changed = False
            i = 0
            while i + 3 < len(insts):
                a, b, c, d = insts[i:i + 4]
                if (
                    type(a).__name__ == "InstLdweights"
                    and type(b).__name__ == "InstMatmult"
                    and type(c).__name__ == "InstLdweights"
                    and type(d).__name__ == "InstMatmult"
                    and b.tile_position is not None
                    and c.tile_position is not None
                    and b.tile_position[0] == 0
                    and c.tile_position[0] == 64
                    and b.tile_size is not None
                    and b.tile_size[0] <= 64
                ):
                    insts[i + 1], insts[i + 2] = c, b
                    changed = True
                    i += 4
                else:
                    i += 1
            if changed:
                blk.instructions = insts


def _legalize_waits(nc):
    """Hoist multi-waits off engine instructions onto preceding NoOps.

    Most walrus instruction encodings (S3_LW matmul, DMA, ACT, DVE, drain)
    only have room for a single sync-wait command; Tile freely attaches
    several. Waits execute on the engine's sequencer in program order, so
    moving them to immediately-preceding NoOps is semantics-preserving.
    """
    import bass_rust

    for f in nc.m.functions:
        for blk in f.blocks:
            out = []
            changed = False
            for inst in blk.instructions:
                si = getattr(inst, "sync_info", None)
                if (
                    type(inst).__name__ not in _NO_HOIST_TYPES
                    and si is not None
                    and len(si.on_wait) >= 2
                ):
                    waits = list(si.on_wait)
                    for k, w in enumerate(waits[:-1]):
                        nop = bass_rust.InstNoOp(name=f"{inst.name}_hoistw{k}")
                        nop.engine = inst.engine
                        nop.sync_info = bass_rust.SyncInfo(
                            on_wait=[w], on_update=[]
                        )
                        out.append(nop)
                    si.on_wait = [waits[-1]]
                    changed = True
                out.append(inst)
            if changed:
                blk.instructions = out


def _build_program():
    import concourse.bass as bass
    import concourse.mybir as mybir
    import concourse.tile as tile

    nc = bass.Bass("TRN2", target_bir_lowering=False, debug=False, num_devices=NCORES)
    xT = nc.dram_tensor("xT", [D, T], mybir.dt.bfloat16, kind="ExternalInput").ap()
    w6 = nc.dram_tensor("w6", [D, 6 * HS], mybir.dt.bfloat16, kind="ExternalInput").ap()
    out = nc.dram_tensor("outR", [HPC * (HS + 1), T], mybir.dt.float32, kind="ExternalOutput").ap()

    with tile.TileContext(nc) as tc:
        _emit(tc, nc, xT, w6, out)
    _pair_ldweights(nc)
    _legalize_waits(nc)
    return nc


def _in_maps(x, Wk, Wq, Wv):
    import ml_dtypes

    bf = ml_dtypes.bfloat16
    xTh = np.ascontiguousarray(np.asarray(x, dtype=np.float32).T.astype(bf))
    maps = []
    for c in range(NCORES):
        h0, h1 = HPC * c, HPC * c + 1
        W6 = np.concatenate(
            [Wq[h0], Wq[h1], Wk[h0], Wk[h1], Wv[h0], Wv[h1]], axis=1
        ).astype(bf)
        maps.append({"xT": xTh, "w6": np.ascontiguousarray(W6)})
    return maps


def get_program():
    global _cached_nc
    if _cached_nc is None:
        _cached_nc = _build_program()
    return _cached_nc


def kernel(x, Wk, Wq, Wv):
    import os

    from concourse.bass_utils import run_bass_kernel_spmd

    # The neuronx-cc compile cache keys on tensor shapes only (not BIR
    # content), so a shared cache can serve a stale NEFF for a same-shape
    # program. Force a fresh compile; repeat calls in one process still hit
    # the in-memory jit cache.
    os.environ.setdefault("NEURON_FORCE_RECOMPILE", "1")

    nc = get_program()
    in_maps = _in_maps(x, Wk, Wq, Wv)
    # Warmup execution: the first run after a fresh compile/device-load has
    # been observed to occasionally return corrupted output; run twice and
    # use the second result (costs ~1s of host wall time, no HW-time impact).
    run_bass_kernel_spmd(nc, in_maps, core_ids=list(range(NCORES)))
    res = run_bass_kernel_spmd(nc, in_maps, core_ids=list(range(NCORES)))
    cols = []
    for c in range(NCORES):
        raw = res.results[c]["outR"]  # [2*65, T]: per head 64 rows O^T + denom
        for h in range(HPC):
            o = raw[h * 65:h * 65 + HS]
            den = raw[h * 65 + HS:h * 65 + HS + 1]
            cols.append((o / den).T)
    return np.ascontiguousarray(np.concatenate(cols, axis=1), dtype=np.float32)


# revision 20
# speedup vs baseline: 1.1775x; 1.1775x over previous
"""Multi-head causal attention (scores = K @ Q^T variant) on 8 TRN2 NeuronCores.

Head-parallel sharding: core c computes heads (2c, 2c+1) end-to-end and the
host concatenates the per-core [T, 128] outputs along the feature axis.

The baseline was Scalar(ACT)-bound: one exp per 128-key j-block paced the
whole steady state at ~1.07us, with the 2-deep score-PSUM ring
(score(jb+2) waits exp(jb)) locking the PE to the exp engine.  This version
splits the softmax exp across engines in strictly alternating key-block
pairs and uses fp8 DoubleRow for the ACT-path AV matmuls:

  - "f8" pairs (alternating off-diagonal): ACT exp emits fp8e4 directly
    (exp(s/8 - OFF), OFF=2 keeps exp below the e4m3 max; the offset divides
    out of the softmax ratio exactly).  The AV contraction runs as ONE
    DoubleRow matmul per head per 256-key pair (2 fp8 weights/cell, virtual
    256-deep contraction) instead of 4 bf16 matmuls.
  - "dve" pairs (the other off-diagonal half + first diagonal pair): the
    Vector engine computes a Schraudolph fast-exp: int16((s + B/A) * A) IS
    the bf16 bit pattern of e^(s/8-OFF) with a piecewise-linear mantissa
    (~+-3% after centering, cancels in the softmax ratio).  On diagonal
    blocks the causal mask folds into the same instruction: the in1 scale
    tensor holds A where kept and 0 where masked, so masked lanes emit
    int16 0 == bf16 +0.0.
  - "a16" (second diagonal pair + all of i-block 0): exact ACT exp in bf16
    + mask multiply — rows with few keys get no sawtooth/quantization error
    (they lack the averaging that damps it elsewhere).

The strict dve/ACT pair alternation gives each exp engine a two-pair
(~3.8us) window per ~2.2us of exp work, so the PSUM ring never binds; AV
matmuls are emitted two pairs behind the scores in whole-pair groups, which
keeps same-geometry matmuls back-to-back on the PE (only the first matmul
after a weight-geometry change pays the ~110ns LDWEIGHTS exposure).

Numerics (numpy sim vs fp64 reference, matches HW): rel absmax ~1.0e-2 vs
the 2e-2 gate.  Measured: ~196-199us vs the 214.6us bf16 baseline.

Other notes (unchanged from baseline): x arrives transposed/bf16; Q^T/K^T
live [128, T] with the two heads on disjoint 64-row PE groups so their score
matmuls run concurrently; scores are computed transposed so AV contracts on
the partition axis; V carries a fused ones-column so one matmul yields both
the weighted sum and the softmax denominator; the host does the final divide
and un-transpose during the gather.
"""

import numpy as np

T, D, H, HS = 4096, 1024, 16, 64
NCORES = 8
HPC = H // NCORES  # heads per core = 2
DC = D // 128      # 8 contraction chunks
TC = T // 512      # 8 t-chunks for projections
JBN = T // 128     # 32 j-blocks (128 keys each)
VF_W = 80          # fp8 V row stride (65 used; 80 keeps DoubleRow step%16==0)

OFF = 2.0                       # exp offset: exp(s/8 - OFF); softmax-invariant
LOG2E = 1.4426950408889634
FE_A = 128.0 * LOG2E / 8.0      # fast-exp scale on raw scores
FE_C = 0.0430                   # centers the PWL sawtooth (half of 0.0861)
FE_B = 128.0 * (127.0 - FE_C - OFF * LOG2E)

_cached_nc = None


def _kind(ib, p):
    """Exp path for key-block pair p (j-blocks 2p, 2p+1) of i-block ib.

    Pairs strictly alternate between the DVE fast-exp path and the ACT
    paths (fp8 off-diagonal / exact-bf16 a16): the 2-deep score-PSUM ring
    makes score(jb+2) wait on exp(jb), so two consecutive same-engine pairs
    lock the pipeline to that engine's ~1.1us-per-jb exp latency, while
    alternation gives each engine a two-pair window per pair of exps.
    Sequence per block: dve, f8, dve, f8, ..., f8, dve(diag), a16(diag).
    ib0 is entirely exact (a16) to protect the few-key early rows.
    """
    npair = 2 * (ib + 1)
    if ib == 0:
        return "a16"
    if p == npair - 2:
        return "dve"   # diagonal blocks q=0,1: mask folds into the fast-exp
    if p == npair - 1:
        return "a16"   # diagonal blocks q=2,3: exact exp + mask multiply
    return "f8" if p % 2 == 1 else "dve"


def _emit(tc, nc, xT, w6, out):
    import concourse.bass as bass  # noqa: F401
    import concourse.mybir as mybir

    f32 = mybir.dt.float32
    bf16 = mybir.dt.bfloat16
    fp8 = mybir.dt.float8e4
    i16 = mybir.dt.int16
    Exp = mybir.ActivationFunctionType.Exp
    CopyF = mybir.ActivationFunctionType.Copy
    ne = mybir.AluOpType.not_equal
    ge = mybir.AluOpType.is_ge
    add = mybir.AluOpType.add
    mult = mybir.AluOpType.mult
    DR = mybir.MatmulPerfMode.DoubleRow

    with (
        tc.tile_pool(name="const", bufs=1) as constp,
        tc.tile_pool(name="wpool", bufs=1) as wpool,
        tc.tile_pool(name="bigp", bufs=1) as bigp,
        tc.tile_pool(name="xpool", bufs=3) as xpool,
        tc.tile_pool(name="vtp", bufs=2) as vtp,
        tc.tile_pool(name="esb", bufs=12) as esbp,
        tc.tile_pool(name="esf", bufs=6) as esfp,
        tc.tile_pool(name="finp", bufs=4) as finp,
        # PSUM budget (8 banks): s 2x2 + o 2x1 + p 2x1 (p shared with the
        # V-transpose tiles).
        tc.tile_pool(name="sp", bufs=2, space="PSUM") as sp,
        tc.tile_pool(name="op", bufs=2, space="PSUM") as op,
        tc.tile_pool(name="pp", bufs=2, space="PSUM") as pp,
    ):
        # ---- input DMAs for weights + first x chunk go first ------------
        w6r = w6.rearrange("(dc p) f -> p dc f", p=128)
        xTr = xT.rearrange("(dc p) t -> p dc t", p=128)
        w6sb = wpool.tile([128, DC, 6 * HS], bf16)
        xts = []
        xt0 = xpool.tile([128, DC, 512], bf16, tag="xt", name="xt0")
        nc.sync.dma_start(out=w6sb[:, 0, :], in_=w6r[:, 0, :])
        nc.sync.dma_start(out=xt0[:, 0, :], in_=xTr[:, 0, 0:512])
        nc.sync.dma_start(out=w6sb[:, 1:DC, :], in_=w6r[:, 1:DC, :])
        for dc in range(1, DC):
            nc.sync.dma_start(out=xt0[:, dc, :], in_=xTr[:, dc, 0:512])
        xts.append(xt0)

        # ---- constants (gpsimd; overlaps the DMAs) ----------------------
        id64 = constp.tile([128, 64], bf16)
        nc.gpsimd.memset(id64, 0.0)
        nc.gpsimd.affine_select(
            out=id64, in_=id64, compare_op=ne, fill=1.0,
            base=0, channel_multiplier=1, pattern=[[-1, 64]],
        )
        nc.gpsimd.affine_select(
            out=id64, in_=id64, compare_op=ne, fill=1.0,
            base=-64, channel_multiplier=1, pattern=[[-1, 64]],
        )
        # bf16 multiplicative causal masks (a16 path): keep iff il >= jl + 128q.
        mask4 = constp.tile([128, 4, 512], bf16)
        for q in range(4):
            nc.gpsimd.memset(mask4[:, q, :], 1.0)
            nc.gpsimd.affine_select(
                out=mask4[:, q, :], in_=mask4[:, q, :], compare_op=ge, fill=0.0,
                base=-128 * q, channel_multiplier=-1, pattern=[[1, 512]],
            )
        # fp32 fast-exp scale masks (dve path): FE_A where kept, 0 masked.
        amask4 = constp.tile([128, 4, HPC, 512], f32)
        for q in range(4):
            nc.gpsimd.memset(amask4[:, q, :, :], FE_A)
            for h in range(HPC):
                nc.gpsimd.affine_select(
                    out=amask4[:, q, h, :], in_=amask4[:, q, h, :],
                    compare_op=ge, fill=0.0,
                    base=-128 * q, channel_multiplier=-1, pattern=[[1, 512]],
                )

        # ---- persistent activations ------------------------------------
        QT = bigp.tile([128, T], bf16)   # head0 rows 0-63, head1 rows 64-127
        KT = bigp.tile([128, T], bf16)
        Vb = [bigp.tile([128, JBN, HS + 1], bf16, name=f"Vb{h}") for h in range(HPC)]
        # fp8 V for the DoubleRow AV pairs: slot [p, par] holds key-block
        # 2p + par, so any pair p can run the fp8 path.
        Vf = [bigp.tile([128, JBN // 2, 2, VF_W], fp8, name=f"Vf{h}") for h in range(HPC)]
        onesb = constp.tile([128, JBN], bf16)
        nc.gpsimd.memset(onesb, 1.0)
        for h in range(HPC):
            nc.vector.tensor_copy(Vb[h][:, :, HS], onesb)
            nc.gpsimd.tensor_copy(Vf[h][:, :, :, HS], onesb)
        boff = constp.tile([128, 1], f32)  # activation bias: exp(s/8 - OFF)
        nc.gpsimd.memset(boff, -OFF)

        def emit_proj_chunk(tcj):
            ts = slice(tcj * 512, (tcj + 1) * 512)
            if tcj + 1 < TC:  # prefetch next x chunk (batched 3D DMAs)
                nxt = slice((tcj + 1) * 512, (tcj + 2) * 512)
                xtn = xpool.tile([128, DC, 512], bf16, tag="xt", name=f"xt{tcj + 1}")
                nc.sync.dma_start(out=xtn[:, 0:4, :], in_=xTr[:, 0:4, nxt])
                nc.sync.dma_start(out=xtn[:, 4:8, :], in_=xTr[:, 4:8, nxt])
                xts.append(xtn)
            xt = xts[tcj]
            for fc, dest in ((0, QT), (1, KT)):
                ps = pp.tile([128, 512], f32, tag="p", name=f"ps{fc}_{tcj}")
                for dc in range(DC):
                    nc.tensor.matmul(
                        ps,
                        lhsT=w6sb[:, dc, fc * 128:(fc + 1) * 128],
                        rhs=xt[:, dc, :],
                        start=(dc == 0), stop=(dc == DC - 1),
                    )
                nc.scalar.activation(dest[:, ts], ps, CopyF, scale=1.0)
            psv = pp.tile([128, 512], f32, tag="p", name=f"psv_{tcj}")
            for dc in range(DC):
                nc.tensor.matmul(
                    psv,
                    lhsT=w6sb[:, dc, 256:384],
                    rhs=xt[:, dc, :],
                    start=(dc == 0), stop=(dc == DC - 1),
                )
            def vwork():
                # Deferred into the attention block (after pair 1): emitted
                # here, the vts CAST would sit in DVE's FIFO ahead of the
                # block's first ring-critical fast-exp and stall the PE
                # ~925ns per chunk.  The transposed V is only needed by the
                # block's diagonal pairs, many microseconds later.
                vts = vtp.tile([128, 512], bf16, tag="vts", name=f"vts_{tcj}")
                nc.vector.tensor_copy(vts, psv)
                for q in range(4):
                    jb = tcj * 4 + q
                    for h in range(HPC):
                        ptv = pp.tile([128, 64], bf16, tag="p",
                                      name=f"ptv{h}_{tcj}_{q}")
                        nc.tensor.transpose(
                            ptv,
                            in_=vts[h * 64:(h + 1) * 64, q * 128:(q + 1) * 128],
                            identity=id64[h * 64:(h + 1) * 64, :],
                            tile_position=(h * 64, 0),
                        )
                        nc.vector.tensor_copy(Vb[h][:, jb, 0:HS], ptv)
                        nc.gpsimd.tensor_copy(
                            Vf[h][:, jb // 2, jb % 2, 0:HS],
                            Vb[h][:, jb, 0:HS],
                        )
            return vwork

        def emit_attn_block(ib, vwork=None):
            isl = slice(ib * 512, (ib + 1) * 512)
            npair = 2 * (ib + 1)
            po = [
                op.tile([HS + 1, 512], f32, tag="o", name=f"po{h}_{ib}")
                for h in range(HPC)
            ]

            def emit_av(p, kind, es, first, last):
                if kind == "f8":
                    for h in range(HPC):
                        nc.tensor.matmul(
                            po[h],
                            lhsT=Vf[h][:, p, :, 0:HS + 1],
                            rhs=es[:, :, h, :],
                            start=first, stop=last,
                            perf_mode=DR,
                        )
                else:
                    for t in range(2):
                        for h in range(HPC):
                            nc.tensor.matmul(
                                po[h],
                                lhsT=Vb[h][:, 2 * p + t, :],
                                rhs=es[t][:, h, :],
                                start=(first and t == 0),
                                stop=(last and t == 1),
                            )

            pending = []
            for p in range(npair):
                kind = _kind(ib, p)
                if kind == "f8":
                    es = esfp.tile(
                        [128, 2, HPC, 512], fp8, tag="es8", name=f"es8_{ib}_{p}"
                    )
                else:
                    es = [
                        esbp.tile([128, HPC, 512], bf16, tag="esb",
                                  name=f"esb_{ib}_{p}_{t}")
                        for t in range(2)
                    ]
                for t in range(2):
                    jb = 2 * p + t
                    ps = sp.tile([128, HPC, 512], f32, tag="s",
                                 name=f"s_{ib}_{jb}")
                    for h in range(HPC):
                        nc.tensor.matmul(
                            ps[:, h, :],
                            lhsT=QT[h * 64:(h + 1) * 64, jb * 128:(jb + 1) * 128],
                            rhs=KT[h * 64:(h + 1) * 64, isl],
                            start=True, stop=True,
                            tile_position=(h * 64, 0),
                        )
                    q = jb - 4 * ib  # diagonal offset (>=0 on diagonal blocks)
                    if kind == "f8":
                        nc.scalar.activation(
                            es[:, t, :, :], ps, Exp,
                            scale=float(1.0 / np.sqrt(HS)), bias=boff,
                        )
                    elif kind == "a16":
                        nc.scalar.activation(
                            es[t], ps, Exp,
                            scale=float(1.0 / np.sqrt(HS)), bias=boff,
                        )
                        if q >= 0:
                            for h in range(HPC):
                                nc.vector.tensor_mul(
                                    es[t][:, h, :], es[t][:, h, :], mask4[:, q, :]
                                )
                    else:  # dve fast-exp; mask folds in via amask4
                        eo = es[t].bitcast(i16)
                        if q >= 0:
                            nc.vector.scalar_tensor_tensor(
                                out=eo, in0=ps, scalar=FE_B / FE_A,
                                in1=amask4[:, q, :, :], op0=add, op1=mult,
                            )
                        else:
                            nc.vector.tensor_scalar(
                                eo, ps, FE_B / FE_A, FE_A, add, mult
                            )
                pending.append((p, kind, es, p == 0, p == npair - 1))
                if len(pending) > 2:
                    emit_av(*pending.pop(0))
                if p == 1 and vwork is not None:
                    vwork()
            for item in pending:
                emit_av(*item)
            # evacuate the unnormalized O^T + denominator row; the host does
            # the (tiny) divide and the un-transpose during the gather.
            for h in range(HPC):
                ot = finp.tile([HS + 1, 512], f32, tag="ot", name=f"ot{h}_{ib}")
                nc.scalar.activation(ot, po[h], CopyF, scale=1.0)
                nc.sync.dma_start(
                    out=out[h * (HS + 1):(h + 1) * (HS + 1), isl], in_=ot
                )

        # Staircase: attention block k only depends on projection chunks <= k.
        for k in range(TC):
            vw = emit_proj_chunk(k)
            if k == 0:
                vw()          # ib0's own diagonal pairs need V immediately
                emit_attn_block(k)
            else:
                emit_attn_block(k, vwork=vw)


# walrus engine-instruction encodings have a single sync-wait slot; hoist
# extra waits onto per-wait NoOps for everything except generated NoOps.
_NO_HOIST_TYPES = frozenset({"InstNoOp"})


def _pair_ldweights(nc):
    """Reorder LDW0,MM0,LDW1,MM1 -> LDW0,LDW1,MM0,MM1 for row-group pairs.

    When the second weight load targets PE rows 64-127 while the first
    matmul only occupies rows 0-63, the loads run concurrently on disjoint
    sub-arrays and both matmul streams overlap, instead of serializing the
    second load behind the first stream.
    """
    for f in nc.m.functions:
        for blk in f.blocks:
            insts = blk.instructions
            changed = False
            i = 0
            while i + 3 < len(insts):
                a, b, c, d = insts[i:i + 4]
                if (
                    type(a).__name__ == "InstLdweights"
                    and type(b).__name__ == "InstMatmult"
                    and type(c).__name__ == "InstLdweights"
                    and type(d).__name__ == "InstMatmult"
                    and b.tile_position is not None
                    and c.tile_position is not None
                    and b.tile_position[0] == 0
                    and c.tile_position[0] == 64
                    and b.tile_size is not None
                    and b.tile_size[0] <= 64
                ):
                    insts[i + 1], insts[i + 2] = c, b
                    changed = True
                    i += 4
                else:
                    i += 1
            if changed:
                blk.instructions = insts


def _legalize_waits(nc):
    """Hoist multi-waits off engine instructions onto preceding NoOps.

    Most walrus instruction encodings (S3_LW matmul, DMA, ACT, DVE, drain)
    only have room for a single sync-wait command; Tile freely attaches
    several. Waits execute on the engine's sequencer in program order, so
    moving them to immediately-preceding NoOps is semantics-preserving.
    """
    import bass_rust

    for f in nc.m.functions:
        for blk in f.blocks:
            out = []
            changed = False
            for inst in blk.instructions:
                si = getattr(inst, "sync_info", None)
                if (
                    type(inst).__name__ not in _NO_HOIST_TYPES
                    and si is not None
                    and len(si.on_wait) >= 2
                ):
                    waits = list(si.on_wait)
                    for k, w in enumerate(waits[:-1]):
                        nop = bass_rust.InstNoOp(name=f"{inst.name}_hoistw{k}")
                        nop.engine = inst.engine
                        nop.sync_info = bass_rust.SyncInfo(
                            on_wait=[w], on_update=[]
                        )
                        out.append(nop)
                    si.on_wait = [waits[-1]]
                    changed = True
                out.append(inst)
            if changed:
                blk.instructions = out


def _build_program():
    import concourse.bass as bass
    import concourse.mybir as mybir
    import concourse.tile as tile

    nc = bass.Bass("TRN2", target_bir_lowering=False, debug=False, num_devices=NCORES)
    xT = nc.dram_tensor("xT", [D, T], mybir.dt.bfloat16, kind="ExternalInput").ap()
    w6 = nc.dram_tensor("w6", [D, 6 * HS], mybir.dt.bfloat16, kind="ExternalInput").ap()
    out = nc.dram_tensor("outR", [HPC * (HS + 1), T], mybir.dt.float32, kind="ExternalOutput").ap()

    with tile.TileContext(nc) as tc:
        _emit(tc, nc, xT, w6, out)
    _pair_ldweights(nc)
    _legalize_waits(nc)
    return nc


def _in_maps(x, Wk, Wq, Wv):
    import ml_dtypes

    bf = ml_dtypes.bfloat16
    xTh = np.ascontiguousarray(np.asarray(x, dtype=np.float32).T.astype(bf))
    maps = []
    for c in range(NCORES):
        h0, h1 = HPC * c, HPC * c + 1
        W6 = np.concatenate(
            [Wq[h0], Wq[h1], Wk[h0], Wk[h1], Wv[h0], Wv[h1]], axis=1
        ).astype(bf)
        maps.append({"xT": xTh, "w6": np.ascontiguousarray(W6)})
    return maps


def get_program():
    global _cached_nc
    if _cached_nc is None:
        _cached_nc = _build_program()
    return _cached_nc


def kernel(x, Wk, Wq, Wv):
    import os

    from concourse.bass_utils import run_bass_kernel_spmd

    # The neuronx-cc compile cache keys on tensor shapes only (not BIR
    # content), so a shared cache can serve a stale NEFF for a same-shape
    # program. Force a fresh compile; repeat calls in one process still hit
    # the in-memory jit cache.
    os.environ.setdefault("NEURON_FORCE_RECOMPILE", "1")

    nc = get_program()
    in_maps = _in_maps(x, Wk, Wq, Wv)
    # Warmup execution: the first run after a fresh compile/device-load has
    # been observed to occasionally return corrupted output; run twice and
    # use the second result (costs ~1s of host wall time, no HW-time impact).
    run_bass_kernel_spmd(nc, in_maps, core_ids=list(range(NCORES)))
    res = run_bass_kernel_spmd(nc, in_maps, core_ids=list(range(NCORES)))
    cols = []
    for c in range(NCORES):
        raw = res.results[c]["outR"]  # [2*65, T]: per head 64 rows O^T + denom
        for h in range(HPC):
            o = raw[h * 65:h * 65 + HS]
            den = raw[h * 65 + HS:h * 65 + HS + 1]
            cols.append((o / den).T)
    return np.ascontiguousarray(np.concatenate(cols, axis=1), dtype=np.float32)
